# revision 29
# baseline (speedup 1.0000x reference)
"""Multi-head attention (B=4, T=2048, D=1024, H=16 causal) on 8 TRN2 NeuronCores.

Sharding: core c -> (batch b = c // 2, head-group g = c % 2 of 8 heads).
Device computes, per core, in transposed layouts (feature-major):
  qT/kT = (W_{q,k,g} @ X_b^T + b)   -- per 512-col stripe tiles, fp16
  V     = X_b @ W_{v,g}^T           -- (2048, 8, 64) natural layout + 64
                                       ones-cols per head (replicated-Z)
  S^T   = per 128-row k-block vs qT -- both heads of a pair packed as
          concurrent 64x128 row-tiled matmuls (PE rows 0-63 / 64-127)
          into one [128, 2, 512] psum tile (bank A / bank B)
  P~    = exp(S^T/8)  one ACT instr per k-block covering both heads
  [O^T; Z*64] = [V|1x64]^T @ P~     -- ones block gives softmax denom
          replicated on po rows 64..127, so 1/Z comes from a single
          64-lane reciprocal_approx_fast straight off PSUM (no 1-lane
          copies, no GpSimd partition-broadcast)
  O^T  = po[0:64] * rb, then Y_part = O^T-chunks^T @ W_o-slice^T.

Single software pipeline, kt-granular (288us -> target ~250us):
  - score-PSUM tiles are per-k-block and hold BOTH heads, so the two
    64x128 score matmuls are emitted adjacently with identical deps and
    the PE runs them concurrently on disjoint row groups (~2x scores);
    the single-tile release (one exp per k-block) keeps both heads'
    slots freeing together, which is what lets the next block pack too;
  - a ~4us warm-up spinner of tiny matmuls at kernel start trips the
    HAM clock gate to 8/8 before the DMA-bound head delivers real work
    (baseline only warmed at ~33us, paying 2x on all head matmuls);
  - projection bias-evac runs on the DVE (tensor_scalar_add), keeping
    the scalar engine for the ~157us exp stream;
  - output projection for stripes 0..2 is deferred and drained through
    the final stripe as PE filler; the final stripe's own blocks drain
    at the end.
PSUM budget: ss 2x2 (two [128,2,512] slots) + po 2 + pp 2 = 8 banks.
Host: shards/transposes inputs, sums the two per-batch partial Y's,
adds b_o plus the folded V-bias constant row (b_v,g @ W_o,g^T).
"""
import numpy as np
from contextlib import ExitStack

B, T, D = 4, 2048, 1024
H, DK = 16, 64
NCORES = 8
HPC = H // 2            # heads per core
F = HPC * DK            # 512 features per core
SCALE = 1.0 / np.sqrt(DK)
TQ = 512                # q-tile width (free dim)
TK = 128                # k-tile height (partition dim)
NQT = T // TQ           # 4
NKT = T // TK           # 16
ND = D // 128           # 8 contraction chunks for projections
NF = F // 128           # 4 feature chunks per core
PT = 512                # projection t-tile width (== TQ)
NPT = T // PT           # 4

_compiled = {}


def _build(causal: bool):
    import concourse.tile as tile
    from concourse import bacc, mybir

    dt = mybir.dt
    AF = mybir.ActivationFunctionType
    ALU = mybir.AluOpType

    nc = bacc.Bacc("TRN2", target_bir_lowering=False, debug=False,
                   num_devices=NCORES)

    xq = nc.dram_tensor("xq", [D, T], dt.float8e4, kind="ExternalInput")
    xk = nc.dram_tensor("xk", [D, T], dt.float8e4, kind="ExternalInput")
    xv = nc.dram_tensor("xv", [D, T], dt.float16, kind="ExternalInput")
    wq = nc.dram_tensor("wq", [D, F], dt.float8e4, kind="ExternalInput")
    wk = nc.dram_tensor("wk", [D, F], dt.float8e4, kind="ExternalInput")
    wv = nc.dram_tensor("wv", [D, F], dt.float16, kind="ExternalInput")
    wo = nc.dram_tensor("wo", [F, D], dt.float16, kind="ExternalInput")
    bq = nc.dram_tensor("bq", [128, NF], dt.float32, kind="ExternalInput")
    bk = nc.dram_tensor("bk", [128, NF], dt.float32, kind="ExternalInput")
    tri = nc.dram_tensor("tri", [128, 128], dt.float16, kind="ExternalInput")
    y = nc.dram_tensor("y", [T, D], dt.float16, kind="ExternalOutput")

    with tile.TileContext(nc) as tc, ExitStack() as ctx:
        per = ctx.enter_context(tc.tile_pool(name="persist", bufs=1))

        # per-stripe persistent tiles -> fine-grained scheduler deps
        qT = [[per.tile([128, PT], dt.float16, tag=f"qT{f}_{t}",
                        name=f"qT{f}_{t}")
               for t in range(NPT)] for f in range(NF)]
        kT = [[per.tile([128, PT], dt.float16, tag=f"kT{f}_{t}",
                        name=f"kT{f}_{t}")
               for t in range(NPT)] for f in range(NF)]
        # V with 64 leading ones-cols per head: PV output rows 0..63 = Z
        # (replicated), rows 64..127 = O^T; recip needs partition base 0
        vS = [per.tile([128, HPC, 2 * DK], dt.float16, tag=f"v{t}",
                       name=f"v{t}")
              for t in range(NKT)]
        oT = [[per.tile([128, TQ], dt.float16, tag=f"oT{f}_{t}",
                        name=f"oT{f}_{t}")
               for t in range(NQT)] for f in range(NF)]
        bq_sb = per.tile([128, NF], dt.float32, tag="bq")
        bk_sb = per.tile([128, NF], dt.float32, tag="bk")
        tri_sb = per.tile([128, 128], dt.float16, tag="tri")
        warm_sb = per.tile([128, 128], dt.float16, tag="warm")

        # q/k weights in one tile, DMA'd in f-column slices so the first
        # attention unit's chain (wq_f0, xq, wk_f0, xk) is ~2.5MB instead
        # of 4MB; v weights in two halves (512KB per issue)
        wq_sb = per.tile([128, ND, F], dt.float8e4, tag="wq", name="wq")
        wk_sb = per.tile([128, ND, F], dt.float8e4, tag="wk", name="wk")
        wv_sb = per.tile([128, ND, F], dt.float16, tag="wv", name="wv")
        wo_sb = per.tile([128, NF, D], dt.float16, tag="wo")

        wq_re = wq.ap().rearrange("(c p) f -> p c f", p=128)
        wk_re = wk.ap().rearrange("(c p) f -> p c f", p=128)
        wv_re = wv.ap().rearrange("(c p) f -> p c f", p=128)
        xq_re = xq.ap().rearrange("(c p) t -> p c t", p=128)
        xk_re = xk.ap().rearrange("(c p) t -> p c t", p=128)
        xv_re = xv.ap().rearrange("(c p) t -> p c t", p=128)

        px = ctx.enter_context(tc.tile_pool(name="px", bufs=2))
        pps = ctx.enter_context(tc.tile_pool(name="pps", bufs=2, space="PSUM"))
        pa = ctx.enter_context(tc.tile_pool(name="pa", bufs=3))
        pn = ctx.enter_context(tc.tile_pool(name="pn", bufs=2))
        sps = ctx.enter_context(tc.tile_pool(name="sps", bufs=2, space="PSUM"))
        ops = ctx.enter_context(tc.tile_pool(name="ops", bufs=1, space="PSUM"))

        # ---- PE warm-up spinner -----------------------------------------
        # ~4us of continuous N=128 matmuls starting right after the ~7.2us
        # engine-init preamble, so the HAM clock gate reaches 8/8 at
        # ~11.5us (instead of ~33us) while the DMA-bound head streams in;
        # 16 disjoint psum regions so Tile inserts no serializing WAW deps.
        nc.vector.memset(warm_sb[:], 0.0)
        warm_ps = pps.tile([128, PT], dt.float32, tag="pp", name="warm")
        for i in range(44):
            # same col position for all (serial ~107ns cadence, NOT
            # col-group packed) so the spin actually spans ~4.7us
            r = 128 * (i % 4)
            nc.tensor.matmul(warm_ps[0:32, r:r + 128],
                             warm_sb[:, 0:32], warm_sb[:, 0:128],
                             start=True, stop=True)

        # ---- projection sub-tasks ----------------------------------------
        def qk_dma(x_re, t):
            xt = px.tile([128, ND, PT], dt.float8e4, tag="xt", name="xt",
                         bufs=2)
            nc.sync.dma_start(xt[:], x_re[:, :, t * PT:(t + 1) * PT])
            return xt

        def qk_chunk(getx, w_sb, b_sb, dest, t, f):
            # fp8 DoubleRow: 4 K=256 matmuls (2 elem/cell/cycle) replace
            # 8 K=128 fp16 matmuls -- ~2x projection throughput
            ps = pps.tile([128, PT], dt.float32, tag="pp", name="pp")
            for dp in range(ND // 2):
                nc.tensor.matmul(
                    ps[:], w_sb[:, 2 * dp:2 * dp + 2, f * 128:(f + 1) * 128],
                    getx(dp), start=(dp == 0), stop=(dp == ND // 2 - 1),
                    perf_mode=mybir.MatmulPerfMode.DoubleRow)
            # bias-add on DVE keeps the scalar engine free for exp
            nc.vector.tensor_scalar_add(dest[f][t][:], ps[:],
                                        b_sb[:, f:f + 1])

        def v_tile(ts):
            xt = px.tile([128, ND, TK], dt.float16, tag="xtv", name="xtv",
                         bufs=3)
            nc.sync.dma_start(xt[:], xv_re[:, :, ts * TK:(ts + 1) * TK])
            ps = pps.tile([128, F], dt.float32, tag="pp", name="pp")
            for d in range(ND):
                nc.tensor.matmul(ps[:], xt[:, d, :],
                                 wv_sb[:, d, :],
                                 start=(d == 0), stop=(d == ND - 1))
            nc.vector.tensor_copy(
                vS[ts][:, :, DK:2 * DK],
                ps[:].rearrange("p (h e) -> p h e", h=HPC))

        # ---- prologue: minimal stripe-0 work for the first attention unit
        # the head is HBM-transfer-bound: issue strictly in consumption
        # order (q/k halves interleaved so the chains pipeline with the
        # transfers), >=512KB per issue; biases (needed only at evac) and
        # v data come after.  All on the sync queue — DMA issues on the
        # scalar queue would block exp behind them.
        for ts in range(NKT):
            nc.vector.memset(vS[ts][:, :, 0:DK], 1.0)
        xq0 = [px.tile([128, ND // 2, PT], dt.float8e4, tag=f"xq0_{g}",
                       name=f"xq0_{g}") for g in range(2)]
        xk0 = [px.tile([128, ND // 2, PT], dt.float8e4, tag=f"xk0_{g}",
                       name=f"xk0_{g}") for g in range(2)]
        # chain to the first attention unit: wq_f0, xq halves, wk_f0, xk
        # halves (~2.5MB -> first exp at ~16us instead of ~27); then v
        # weights + stripe-0 v slices, then the remaining weight f-slices
        # in consumption order
        nc.sync.dma_start(wq_sb[:, :, 0:128], wq_re[:, :, 0:128])
        nc.sync.dma_start(xq0[0][:], xq_re[:, 0:4, 0:PT])
        nc.sync.dma_start(xq0[1][:], xq_re[:, 4:8, 0:PT])
        nc.sync.dma_start(bq_sb[:], bq.ap())
        nc.sync.dma_start(wk_sb[:, :, 0:128], wk_re[:, :, 0:128])
        nc.sync.dma_start(xk0[0][:], xk_re[:, 0:4, 0:PT])
        nc.sync.dma_start(xk0[1][:], xk_re[:, 4:8, 0:PT])
        nc.sync.dma_start(bk_sb[:], bk.ap())
        if causal:
            nc.sync.dma_start(tri_sb[:], tri.ap())
        for g in range(2):
            nc.sync.dma_start(wv_sb[:, 4 * g:4 * g + 4, :],
                              wv_re[:, 4 * g:4 * g + 4])
        nc.sync.dma_start(wq_sb[:, :, 128:256], wq_re[:, :, 128:256])
        nc.sync.dma_start(wk_sb[:, :, 128:256], wk_re[:, :, 128:256])

        def x0pair(x0):
            return lambda dp: x0[dp // 2][:, 2 * (dp % 2):2 * (dp % 2) + 2, :]

        qk_chunk(x0pair(xq0), wq_sb, bq_sb, qT, 0, 0)
        qk_chunk(x0pair(xk0), wk_sb, bk_sb, kT, 0, 0)
        for ts in range(4):
            v_tile(ts)
        nc.sync.dma_start(wq_sb[:, :, 256:384], wq_re[:, :, 256:384])
        nc.sync.dma_start(wk_sb[:, :, 256:384], wk_re[:, :, 256:384])
        nc.sync.dma_start(wq_sb[:, :, 384:512], wq_re[:, :, 384:512])
        nc.sync.dma_start(wk_sb[:, :, 384:512], wk_re[:, :, 384:512])

        # remaining stripe-0 q/k chunks: drained inside qt0, one f ahead
        # of the attention unit that consumes them
        fill_own0 = []
        for f in range(1, NF):
            fill_own0.append(lambda f=f: qk_chunk(
                x0pair(xq0), wq_sb, bq_sb, qT, 0, f))
            fill_own0.append(lambda f=f: qk_chunk(
                x0pair(xk0), wk_sb, bk_sb, kT, 0, f))

        # fill tasks for stripe t: fq (q proj, needed when attention(t)
        # starts) and fkv (k/v proj, needed only by attention(t)'s
        # diagonal blocks, which run last)
        def make_fq(t):
            tasks = []
            state = {}
            def qd():
                state['qxt'] = qk_dma(xq_re, t)
            tasks.append(qd)
            for f in range(NF):
                tasks.append(lambda f=f: qk_chunk(
                    lambda dp: state['qxt'][:, 2 * dp:2 * dp + 2, :],
                    wq_sb, bq_sb, qT, t, f))
            return tasks

        def make_fkv(t):
            tasks = []
            state = {}
            def kd():
                state['kxt'] = qk_dma(xk_re, t)
            tasks.append(kd)
            for f in range(NF):
                tasks.append(lambda f=f: qk_chunk(
                    lambda dp: state['kxt'][:, 2 * dp:2 * dp + 2, :],
                    wk_sb, bk_sb, kT, t, f))
                tasks.append(lambda ts=4 * t + f: v_tile(ts))
            return tasks

        # ---- attention: kt-granular units, heads packed -------------------
        def emit_S_exp(qt, c, kt, oo, w):
            # both heads' 64x128 score matmuls into one [128, 2, TQ] psum
            # tile (h0 -> bank A cols, h1 -> bank B cols): identical deps,
            # adjacent issue, disjoint PE row groups -> they pack
            ss = sps.tile([128, 2, TQ], dt.float32, tag="ss", name="ss")
            for par in range(2):
                base = par * DK
                nc.tensor.matmul(
                    ss[:, par, oo:TQ],
                    kT[c][kt // 4][base:base + DK,
                                   (kt % 4) * TK:(kt % 4 + 1) * TK],
                    qT[c][qt][base:base + DK, oo:TQ],
                    start=True, stop=True)
            pt = pa.tile([128, 2, TQ], dt.float16, tag="pt", name="pt",
                         bufs=6)
            nc.scalar.activation(pt[:, :, oo:TQ], ss[:, :, oo:TQ],
                                 AF.Exp, scale=float(SCALE))
            if causal and oo + TK <= TQ and kt >= 4 * qt:
                for par in range(2):
                    nc.vector.tensor_tensor(
                        pt[:, par, oo:oo + TK], pt[:, par, oo:oo + TK],
                        tri_sb[:], op=ALU.mult)
            return pt

        def emit_O(unit):
            qt, c, kt, oo, pt, first, last, po = unit
            for par in range(2):
                h = 2 * c + par
                nc.tensor.matmul(
                    po[par][:, oo:TQ], vS[kt][:, h, :],
                    pt[:, par, oo:TQ],
                    start=first, stop=last)
            if last:
                for par in range(2):
                    base = par * DK
                    # ones block is FIRST in vS, so Z lands on partitions
                    # 0:63 (reciprocal_approx_fast only works at base 0)
                    # and O on 64:127 (cross-base tensor_tensor is fine)
                    rb = pn.tile([DK, TQ], dt.float32, tag=f"rb{par}",
                                 name=f"rb{par}")
                    nc.vector.reciprocal_approx_fast(
                        out=rb[:], in_=po[par][0:DK, :])
                    nc.vector.tensor_tensor(
                        oT[c][qt][base:base + DK, :],
                        po[par][DK:2 * DK, :], rb[:], op=ALU.mult)
                if c == NF - 1:
                    # defer: output projection is the PE filler that keeps
                    # the clock gate warm through the ACT-bound final
                    # stripe; the last stripe's own blocks also go through
                    # the queue so held-back (dependency-free) blocks drain
                    # first and cover the final normalization-chain wait
                    for tsl in range(TQ // 128):
                        op_q.append(
                            lambda qt=qt, tsl=tsl, **kw:
                            emit_op_block(qt, tsl, **kw))

        def emit_op_block(qt, tsl, scalar_evac=False):
            ts = qt * (TQ // 128) + tsl
            yst = pa.tile([128, D], dt.float16, tag="yst", name="yst")
            for mh in range(2):
                ps = pps.tile([128, 512], dt.float32,
                              tag="pp", name=f"yp{mh}")
                for fc in range(NF):
                    nc.tensor.matmul(
                        ps[:],
                        oT[fc][qt][:, tsl * 128:(tsl + 1) * 128],
                        wo_sb[:, fc, mh * 512:(mh + 1) * 512],
                        start=(fc == 0), stop=(fc == NF - 1))
                if scalar_evac and mh == 0:
                    # final-flush blocks: ACT is idle once the exp stream
                    # ends; splitting evac between ScE and DVE removes the
                    # psum-slot stall that was re-throttling the clock gate
                    nc.scalar.copy(yst[:, mh * 512:(mh + 1) * 512], ps[:])
                else:
                    nc.vector.tensor_copy(yst[:, mh * 512:(mh + 1) * 512],
                                          ps[:])
            nc.sync.dma_start(y.ap()[ts * 128:(ts + 1) * 128, :],
                              yst[:])

        pending = None
        op_q = []
        # fill draining: during attention(qt) run all of stripe qt+1's
        # projection work (every head-chunk c runs its diagonal blocks, so
        # stripe tiles must be complete before attention(qt+1) starts);
        # deferred output-projection blocks drain through the last stripe
        # as PE filler against the clock-gate.
        for qt in range(NQT):
            if qt + 1 < NPT:
                fill_b = make_fq(qt + 1) + make_fkv(qt + 1)
                if qt == 0:
                    fill_b.insert(0, lambda: nc.sync.dma_start(
                        wo_sb[:],
                        wo.ap().rearrange("(c p) m -> p c m", p=128)))
            else:
                fill_b = []
            fill_a = fill_own0 if qt == 0 else []
            if causal:
                # regular k-blocks first: the diagonal blocks need stripe
                # qt's k/v tiles, which may still be projecting (fill_a)
                kts = [(kt, 0) for kt in range(4 * qt)] + \
                      [(4 * qt + j, j * TK) for j in range(4)]
            else:
                kts = [(kt, 0) for kt in range(NKT)]
            n_kts = len(kts)
            n_units = NF * n_kts
            n_units_a = NF * max(n_kts - 3, 1)
            done_u = 0
            done_a = 0
            done_b = 0
            done_op = 0
            for c in range(NF):
                po = {}
                for par in range(2):
                    po[par] = ops.tile([128, TQ], dt.float32,
                                       tag=f"po{par}", name=f"po{par}")
                for ui, (kt, oo) in enumerate(kts):
                    pt = emit_S_exp(qt, c, kt, oo, TQ - oo)
                    unit = (qt, c, kt, oo, pt, ui == 0, ui == n_kts - 1,
                            po)
                    if pending is not None:
                        emit_O(pending)
                    pending = unit
                    done_u += 1
                    want_a = min(len(fill_a), -(-done_u * len(fill_a)
                                                // n_units_a))
                    while done_a < want_a:
                        fill_a[done_a]()
                        done_a += 1
                    want_b = -(-done_u * len(fill_b) // n_units)
                    while done_b < want_b:
                        fill_b[done_b]()
                        done_b += 1
                    if qt == NQT - 1:
                        want_op = done_u * 3 * (TQ // 128) // n_units
                        while done_op < want_op and op_q:
                            op_q.pop(0)()
                            done_op += 1
        if pending is not None:
            emit_O(pending)
        # leftover deferred blocks (old stripes, deps long satisfied) fill
        # the PE while the last unit's normalization chain drains
        while op_q:
            op_q.pop(0)(scalar_evac=True)

    nc.compile()
    return nc


def _get(causal: bool):
    if causal not in _compiled:
        _compiled[causal] = _build(causal)
    return _compiled[causal]


def kernel(q, k, v, mask, w_q, b_q, w_k, b_k, w_v, b_v, w_o, b_o):
    from concourse.bass_utils import run_bass_kernel_spmd

    q = np.asarray(q, dtype=np.float32)
    k = np.asarray(k, dtype=np.float32)
    v = np.asarray(v, dtype=np.float32)
    w_q = np.asarray(w_q, dtype=np.float32)
    w_k = np.asarray(w_k, dtype=np.float32)
    w_v = np.asarray(w_v, dtype=np.float32)
    w_o = np.asarray(w_o, dtype=np.float32)
    b_q = np.asarray(b_q, dtype=np.float32)
    b_k = np.asarray(b_k, dtype=np.float32)
    b_v = np.asarray(b_v, dtype=np.float32)
    b_o = np.asarray(b_o, dtype=np.float32)

    m = np.asarray(mask).reshape(T, T)
    idx = np.arange(T)
    if m.all():
        causal = False
    elif (m == (idx[None, :] <= idx[:, None])).all():
        causal = True
    else:
        raise NotImplementedError("only causal (tril) or full masks supported")

    nc = _get(causal)

    tri_np = np.ascontiguousarray(
        np.asarray(idx[:TK, None] <= idx[None, :TK], dtype=np.float16))

    import ml_dtypes
    f8 = ml_dtypes.float8_e4m3fn
    xq_b = [np.ascontiguousarray(q[b].T.astype(f8)) for b in range(B)]
    xk_b = [np.ascontiguousarray(k[b].T.astype(f8)) for b in range(B)]
    xv_b = [np.ascontiguousarray(v[b].T.astype(np.float16)) for b in range(B)]

    gmaps = []
    for g in range(2):
        sl = slice(g * F, (g + 1) * F)
        gmaps.append({
            "wq": np.ascontiguousarray(w_q[sl, :].T.astype(f8)),
            "wk": np.ascontiguousarray(w_k[sl, :].T.astype(f8)),
            "wv": np.ascontiguousarray(w_v[sl, :].T.astype(np.float16)),
            "wo": np.ascontiguousarray(w_o[:, sl].T.astype(np.float16)),
            "bq": np.ascontiguousarray(b_q[sl].reshape(NF, 128).T),
            "bk": np.ascontiguousarray(b_k[sl].reshape(NF, 128).T),
        })

    in_maps = []
    for c in range(NCORES):
        b, g = c // 2, c % 2
        im = {"xq": xq_b[b], "xk": xk_b[b], "xv": xv_b[b], "tri": tri_np}
        im.update(gmaps[g])
        in_maps.append(im)

    res = run_bass_kernel_spmd(nc, in_maps, core_ids=list(range(NCORES)))

    # constant rows folded out of the device computation
    consts = [b_v[g * F:(g + 1) * F] @ w_o[:, g * F:(g + 1) * F].T
              for g in range(2)]
    add_row = (b_o + consts[0] + consts[1]).astype(np.float32)

    out = np.empty((B, T, D), dtype=np.float32)
    for b in range(B):
        out[b] = (res.results[2 * b]["y"].astype(np.float32)
                  + res.results[2 * b + 1]["y"].astype(np.float32) + add_row)
    return out


# revision 31
# speedup vs baseline: 1.0010x; 1.0010x over previous
"""Multi-head attention (B=4, T=2048, D=1024, H=16 causal) on 8 TRN2 NeuronCores.

Sharding: core c -> (batch b = c // 2, head-group g = c % 2 of 8 heads).
Device computes, per core, in transposed layouts (feature-major):
  qT/kT = (W_{q,k,g} @ X_b^T + b)   -- per 512-col stripe tiles, fp16
  V     = X_b @ W_{v,g}^T           -- (2048, 8, 64) natural layout + 64
                                       ones-cols per head (replicated-Z)
  S^T   = per 128-row k-block vs qT -- both heads of a pair packed as
          concurrent 64x128 row-tiled matmuls (PE rows 0-63 / 64-127)
          into one [128, 2, 512] psum tile (bank A / bank B)
  P~    = exp(S^T/8)  one ACT instr per k-block covering both heads
  [O^T; Z*64] = [V|1x64]^T @ P~     -- ones block gives softmax denom
          replicated on po rows 64..127, so 1/Z comes from a single
          64-lane reciprocal_approx_fast straight off PSUM (no 1-lane
          copies, no GpSimd partition-broadcast)
  O^T  = po[0:64] * rb, then Y_part = O^T-chunks^T @ W_o-slice^T.

Single software pipeline, kt-granular (288us -> target ~250us):
  - score-PSUM tiles are per-k-block and hold BOTH heads, so the two
    64x128 score matmuls are emitted adjacently with identical deps and
    the PE runs them concurrently on disjoint row groups (~2x scores);
    the single-tile release (one exp per k-block) keeps both heads'
    slots freeing together, which is what lets the next block pack too;
  - a ~4us warm-up spinner of tiny matmuls at kernel start trips the
    HAM clock gate to 8/8 before the DMA-bound head delivers real work
    (baseline only warmed at ~33us, paying 2x on all head matmuls);
  - projection bias-evac runs on the DVE (tensor_scalar_add), keeping
    the scalar engine for the ~157us exp stream;
  - output projection for stripes 0..2 is deferred and drained through
    the final stripe as PE filler; the final stripe's own blocks drain
    at the end.
PSUM budget: ss 2x2 (two [128,2,512] slots) + po 2 + pp 2 = 8 banks.
Host: shards/transposes inputs, sums the two per-batch partial Y's,
adds b_o plus the folded V-bias constant row (b_v,g @ W_o,g^T).
"""
import numpy as np
from contextlib import ExitStack

B, T, D = 4, 2048, 1024
H, DK = 16, 64
NCORES = 8
HPC = H // 2            # heads per core
F = HPC * DK            # 512 features per core
SCALE = 1.0 / np.sqrt(DK)
TQ = 512                # q-tile width (free dim)
TK = 128                # k-tile height (partition dim)
NQT = T // TQ           # 4
NKT = T // TK           # 16
ND = D // 128           # 8 contraction chunks for projections
NF = F // 128           # 4 feature chunks per core
PT = 512                # projection t-tile width (== TQ)
NPT = T // PT           # 4

_compiled = {}


def _build(causal: bool):
    import concourse.tile as tile
    from concourse import bacc, mybir

    dt = mybir.dt
    AF = mybir.ActivationFunctionType
    ALU = mybir.AluOpType

    nc = bacc.Bacc("TRN2", target_bir_lowering=False, debug=False,
                   num_devices=NCORES)

    xq = nc.dram_tensor("xq", [D, T], dt.float8e4, kind="ExternalInput")
    xk = nc.dram_tensor("xk", [D, T], dt.float8e4, kind="ExternalInput")
    xv = nc.dram_tensor("xv", [D, T], dt.float16, kind="ExternalInput")
    wq = nc.dram_tensor("wq", [D, F], dt.float8e4, kind="ExternalInput")
    wk = nc.dram_tensor("wk", [D, F], dt.float8e4, kind="ExternalInput")
    wv = nc.dram_tensor("wv", [D, F], dt.float16, kind="ExternalInput")
    wo = nc.dram_tensor("wo", [F, D], dt.float16, kind="ExternalInput")
    bq = nc.dram_tensor("bq", [128, NF], dt.float32, kind="ExternalInput")
    bk = nc.dram_tensor("bk", [128, NF], dt.float32, kind="ExternalInput")
    tri = nc.dram_tensor("tri", [128, 128], dt.float16, kind="ExternalInput")
    y = nc.dram_tensor("y", [T, D], dt.float16, kind="ExternalOutput")

    with tile.TileContext(nc) as tc, ExitStack() as ctx:
        per = ctx.enter_context(tc.tile_pool(name="persist", bufs=1))

        # per-stripe persistent tiles -> fine-grained scheduler deps
        qT = [[per.tile([128, PT], dt.float16, tag=f"qT{f}_{t}",
                        name=f"qT{f}_{t}")
               for t in range(NPT)] for f in range(NF)]
        kT = [[per.tile([128, PT], dt.float16, tag=f"kT{f}_{t}",
                        name=f"kT{f}_{t}")
               for t in range(NPT)] for f in range(NF)]
        # V with 64 leading ones-cols per head: PV output rows 0..63 = Z
        # (replicated), rows 64..127 = O^T; recip needs partition base 0
        vS = [per.tile([128, HPC, 2 * DK], dt.float16, tag=f"v{t}",
                       name=f"v{t}")
              for t in range(NKT)]
        oT = [[per.tile([128, TQ], dt.float16, tag=f"oT{f}_{t}",
                        name=f"oT{f}_{t}")
               for t in range(NQT)] for f in range(NF)]
        bq_sb = per.tile([128, NF], dt.float32, tag="bq")
        bk_sb = per.tile([128, NF], dt.float32, tag="bk")
        tri_sb = per.tile([128, 128], dt.float16, tag="tri")
        warm_sb = per.tile([128, 128], dt.float16, tag="warm")

        # q/k weights in one tile, DMA'd in f-column slices so the first
        # attention unit's chain (wq_f0, xq, wk_f0, xk) is ~2.5MB instead
        # of 4MB; v weights in two halves (512KB per issue)
        wq_sb = per.tile([128, ND, F], dt.float8e4, tag="wq", name="wq")
        wk_sb = per.tile([128, ND, F], dt.float8e4, tag="wk", name="wk")
        wv_sb = per.tile([128, ND, F], dt.float16, tag="wv", name="wv")
        wo_sb = per.tile([128, NF, D], dt.float16, tag="wo")

        wq_re = wq.ap().rearrange("(c p) f -> p c f", p=128)
        wk_re = wk.ap().rearrange("(c p) f -> p c f", p=128)
        wv_re = wv.ap().rearrange("(c p) f -> p c f", p=128)
        xq_re = xq.ap().rearrange("(c p) t -> p c t", p=128)
        xk_re = xk.ap().rearrange("(c p) t -> p c t", p=128)
        xv_re = xv.ap().rearrange("(c p) t -> p c t", p=128)

        px = ctx.enter_context(tc.tile_pool(name="px", bufs=2))
        pps = ctx.enter_context(tc.tile_pool(name="pps", bufs=2, space="PSUM"))
        pa = ctx.enter_context(tc.tile_pool(name="pa", bufs=3))
        pn = ctx.enter_context(tc.tile_pool(name="pn", bufs=2))
        sps = ctx.enter_context(tc.tile_pool(name="sps", bufs=2, space="PSUM"))
        ops = ctx.enter_context(tc.tile_pool(name="ops", bufs=1, space="PSUM"))

        # ---- PE warm-up spinner -----------------------------------------
        # ~4us of continuous N=128 matmuls starting right after the ~7.2us
        # engine-init preamble, so the HAM clock gate reaches 8/8 at
        # ~11.5us (instead of ~33us) while the DMA-bound head streams in;
        # 16 disjoint psum regions so Tile inserts no serializing WAW deps.
        nc.vector.memset(warm_sb[:], 0.0)
        warm_ps = pps.tile([128, PT], dt.float32, tag="pp", name="warm")
        for i in range(44):
            # same col position for all (serial ~107ns cadence, NOT
            # col-group packed) so the spin actually spans ~4.7us
            r = 128 * (i % 4)
            nc.tensor.matmul(warm_ps[0:32, r:r + 128],
                             warm_sb[:, 0:32], warm_sb[:, 0:128],
                             start=True, stop=True)

        # ---- projection sub-tasks ----------------------------------------
        def qk_dma(x_re, t):
            xt = px.tile([128, ND, PT], dt.float8e4, tag="xt", name="xt",
                         bufs=2)
            nc.sync.dma_start(xt[:], x_re[:, :, t * PT:(t + 1) * PT])
            return xt

        def qk_chunk(getx, w_sb, b_sb, dest, t, f):
            # fp8 DoubleRow: 4 K=256 matmuls (2 elem/cell/cycle) replace
            # 8 K=128 fp16 matmuls -- ~2x projection throughput
            ps = pps.tile([128, PT], dt.float32, tag="pp", name="pp")
            for dp in range(ND // 2):
                nc.tensor.matmul(
                    ps[:], w_sb[:, 2 * dp:2 * dp + 2, f * 128:(f + 1) * 128],
                    getx(dp), start=(dp == 0), stop=(dp == ND // 2 - 1),
                    perf_mode=mybir.MatmulPerfMode.DoubleRow)
            # bias-add on DVE keeps the scalar engine free for exp
            nc.vector.tensor_scalar_add(dest[f][t][:], ps[:],
                                        b_sb[:, f:f + 1])

        def v_tile(ts):
            xt = px.tile([128, ND, TK], dt.float16, tag="xtv", name="xtv",
                         bufs=3)
            nc.sync.dma_start(xt[:], xv_re[:, :, ts * TK:(ts + 1) * TK])
            ps = pps.tile([128, F], dt.float32, tag="pp", name="pp")
            for d in range(ND):
                nc.tensor.matmul(ps[:], xt[:, d, :],
                                 wv_sb[:, d, :],
                                 start=(d == 0), stop=(d == ND - 1))
            nc.vector.tensor_copy(
                vS[ts][:, :, DK:2 * DK],
                ps[:].rearrange("p (h e) -> p h e", h=HPC))

        # ---- prologue: minimal stripe-0 work for the first attention unit
        # the head is HBM-transfer-bound: issue strictly in consumption
        # order (q/k halves interleaved so the chains pipeline with the
        # transfers), >=512KB per issue; biases (needed only at evac) and
        # v data come after.  All on the sync queue — DMA issues on the
        # scalar queue would block exp behind them.
        for ts in range(NKT):
            nc.vector.memset(vS[ts][:, :, 0:DK], 1.0)
        xq0 = [px.tile([128, ND // 2, PT], dt.float8e4, tag=f"xq0_{g}",
                       name=f"xq0_{g}") for g in range(2)]
        xk0 = [px.tile([128, ND // 2, PT], dt.float8e4, tag=f"xk0_{g}",
                       name=f"xk0_{g}") for g in range(2)]
        # chain to the first attention unit: wq_f0, xq halves, wk_f0, xk
        # halves (~2.5MB -> first exp at ~16us instead of ~27); then v
        # weights + stripe-0 v slices, then the remaining weight f-slices
        # in consumption order
        nc.sync.dma_start(wq_sb[:, :, 0:128], wq_re[:, :, 0:128])
        nc.sync.dma_start(xq0[0][:], xq_re[:, 0:4, 0:PT])
        nc.sync.dma_start(xq0[1][:], xq_re[:, 4:8, 0:PT])
        nc.sync.dma_start(bq_sb[:], bq.ap())
        nc.sync.dma_start(wk_sb[:, :, 0:128], wk_re[:, :, 0:128])
        nc.sync.dma_start(xk0[0][:], xk_re[:, 0:4, 0:PT])
        nc.sync.dma_start(xk0[1][:], xk_re[:, 4:8, 0:PT])
        nc.sync.dma_start(bk_sb[:], bk.ap())
        if causal:
            nc.sync.dma_start(tri_sb[:], tri.ap())
        # remaining q/k weight f-slices BEFORE the v weights: stripe-0
        # f1..f3 projection chunks are the only PE work available while
        # the v-chain streams, and they keep the clock gate warm
        for f in range(1, NF):
            nc.sync.dma_start(wq_sb[:, :, f * 128:(f + 1) * 128],
                              wq_re[:, :, f * 128:(f + 1) * 128])
            nc.sync.dma_start(wk_sb[:, :, f * 128:(f + 1) * 128],
                              wk_re[:, :, f * 128:(f + 1) * 128])
        for g in range(2):
            nc.sync.dma_start(wv_sb[:, 4 * g:4 * g + 4, :],
                              wv_re[:, 4 * g:4 * g + 4])

        def x0pair(x0):
            return lambda dp: x0[dp // 2][:, 2 * (dp % 2):2 * (dp % 2) + 2, :]

        qk_chunk(x0pair(xq0), wq_sb, bq_sb, qT, 0, 0)
        qk_chunk(x0pair(xk0), wk_sb, bk_sb, kT, 0, 0)
        for ts in range(4):
            v_tile(ts)

        # remaining stripe-0 q/k chunks: drained inside qt0, one f ahead
        # of the attention unit that consumes them
        fill_own0 = []
        for f in range(1, NF):
            fill_own0.append(lambda f=f: qk_chunk(
                x0pair(xq0), wq_sb, bq_sb, qT, 0, f))
            fill_own0.append(lambda f=f: qk_chunk(
                x0pair(xk0), wk_sb, bk_sb, kT, 0, f))

        # fill tasks for stripe t: fq (q proj, needed when attention(t)
        # starts) and fkv (k/v proj, needed only by attention(t)'s
        # diagonal blocks, which run last)
        def make_fq(t):
            tasks = []
            state = {}
            def qd():
                state['qxt'] = qk_dma(xq_re, t)
            tasks.append(qd)
            for f in range(NF):
                tasks.append(lambda f=f: qk_chunk(
                    lambda dp: state['qxt'][:, 2 * dp:2 * dp + 2, :],
                    wq_sb, bq_sb, qT, t, f))
            return tasks

        def make_fkv(t):
            tasks = []
            state = {}
            def kd():
                state['kxt'] = qk_dma(xk_re, t)
            tasks.append(kd)
            for f in range(NF):
                tasks.append(lambda f=f: qk_chunk(
                    lambda dp: state['kxt'][:, 2 * dp:2 * dp + 2, :],
                    wk_sb, bk_sb, kT, t, f))
                tasks.append(lambda ts=4 * t + f: v_tile(ts))
            return tasks

        # ---- attention: kt-granular units, heads packed -------------------
        def emit_S_exp(qt, c, kt, oo, w):
            # both heads' 64x128 score matmuls into one [128, 2, TQ] psum
            # tile (h0 -> bank A cols, h1 -> bank B cols): identical deps,
            # adjacent issue, disjoint PE row groups -> they pack
            ss = sps.tile([128, 2, TQ], dt.float32, tag="ss", name="ss")
            for par in range(2):
                base = par * DK
                nc.tensor.matmul(
                    ss[:, par, oo:TQ],
                    kT[c][kt // 4][base:base + DK,
                                   (kt % 4) * TK:(kt % 4 + 1) * TK],
                    qT[c][qt][base:base + DK, oo:TQ],
                    start=True, stop=True)
            pt = pa.tile([128, 2, TQ], dt.float16, tag="pt", name="pt",
                         bufs=6)
            nc.scalar.activation(pt[:, :, oo:TQ], ss[:, :, oo:TQ],
                                 AF.Exp, scale=float(SCALE))
            if causal and oo + TK <= TQ and kt >= 4 * qt:
                for par in range(2):
                    nc.vector.tensor_tensor(
                        pt[:, par, oo:oo + TK], pt[:, par, oo:oo + TK],
                        tri_sb[:], op=ALU.mult)
            return pt

        def emit_O(unit):
            qt, c, kt, oo, pt, first, last, po = unit
            for par in range(2):
                h = 2 * c + par
                nc.tensor.matmul(
                    po[par][:, oo:TQ], vS[kt][:, h, :],
                    pt[:, par, oo:TQ],
                    start=first, stop=last)
            if last:
                for par in range(2):
                    base = par * DK
                    # ones block is FIRST in vS, so Z lands on partitions
                    # 0:63 (reciprocal_approx_fast only works at base 0)
                    # and O on 64:127 (cross-base tensor_tensor is fine)
                    rb = pn.tile([DK, TQ], dt.float32, tag=f"rb{par}",
                                 name=f"rb{par}")
                    nc.vector.reciprocal_approx_fast(
                        out=rb[:], in_=po[par][0:DK, :])
                    nc.vector.tensor_tensor(
                        oT[c][qt][base:base + DK, :],
                        po[par][DK:2 * DK, :], rb[:], op=ALU.mult)
                if c == NF - 1:
                    # defer: output projection is the PE filler that keeps
                    # the clock gate warm through the ACT-bound final
                    # stripe; the last stripe's own blocks also go through
                    # the queue so held-back (dependency-free) blocks drain
                    # first and cover the final normalization-chain wait
                    for tsl in range(TQ // 128):
                        op_q.append(
                            lambda qt=qt, tsl=tsl, **kw:
                            emit_op_block(qt, tsl, **kw))

        def emit_op_block(qt, tsl, scalar_evac=False):
            ts = qt * (TQ // 128) + tsl
            yst = pa.tile([128, D], dt.float16, tag="yst", name="yst")
            for mh in range(2):
                ps = pps.tile([128, 512], dt.float32,
                              tag="pp", name=f"yp{mh}")
                for fc in range(NF):
                    nc.tensor.matmul(
                        ps[:],
                        oT[fc][qt][:, tsl * 128:(tsl + 1) * 128],
                        wo_sb[:, fc, mh * 512:(mh + 1) * 512],
                        start=(fc == 0), stop=(fc == NF - 1))
                if scalar_evac and mh == 0:
                    # final-flush blocks: ACT is idle once the exp stream
                    # ends; splitting evac between ScE and DVE removes the
                    # psum-slot stall that was re-throttling the clock gate
                    nc.scalar.copy(yst[:, mh * 512:(mh + 1) * 512], ps[:])
                else:
                    nc.vector.tensor_copy(yst[:, mh * 512:(mh + 1) * 512],
                                          ps[:])
            nc.sync.dma_start(y.ap()[ts * 128:(ts + 1) * 128, :],
                              yst[:])

        pending = None
        op_q = []
        # fill draining: during attention(qt) run all of stripe qt+1's
        # projection work (every head-chunk c runs its diagonal blocks, so
        # stripe tiles must be complete before attention(qt+1) starts);
        # deferred output-projection blocks drain through the last stripe
        # as PE filler against the clock-gate.
        for qt in range(NQT):
            if qt + 1 < NPT:
                fill_b = make_fq(qt + 1) + make_fkv(qt + 1)
                if qt == 0:
                    # wo (1MB) is only needed by the deferred output
                    # projection in qt3 -- keep it BEHIND stripe-1 data
                    fill_b.append(lambda: nc.sync.dma_start(
                        wo_sb[:],
                        wo.ap().rearrange("(c p) m -> p c m", p=128)))
            else:
                fill_b = []
            fill_a = fill_own0 if qt == 0 else []
            if causal:
                # regular k-blocks first: the diagonal blocks need stripe
                # qt's k/v tiles, which may still be projecting (fill_a)
                kts = [(kt, 0) for kt in range(4 * qt)] + \
                      [(4 * qt + j, j * TK) for j in range(4)]
            else:
                kts = [(kt, 0) for kt in range(NKT)]
            n_kts = len(kts)
            n_units = NF * n_kts
            n_units_a = NF * max(n_kts - 3, 1)
            done_u = 0
            done_a = 0
            done_b = 0
            done_op = 0
            for c in range(NF):
                po = {}
                for par in range(2):
                    po[par] = ops.tile([128, TQ], dt.float32,
                                       tag=f"po{par}", name=f"po{par}")
                for ui, (kt, oo) in enumerate(kts):
                    pt = emit_S_exp(qt, c, kt, oo, TQ - oo)
                    unit = (qt, c, kt, oo, pt, ui == 0, ui == n_kts - 1,
                            po)
                    if pending is not None:
                        emit_O(pending)
                    pending = unit
                    done_u += 1
                    want_a = min(len(fill_a), -(-done_u * len(fill_a)
                                                // n_units_a))
                    while done_a < want_a:
                        fill_a[done_a]()
                        done_a += 1
                    want_b = -(-done_u * len(fill_b) // n_units)
                    while done_b < want_b:
                        fill_b[done_b]()
                        done_b += 1
                    if qt == NQT - 1:
                        want_op = done_u * 3 * (TQ // 128) // n_units
                        while done_op < want_op and op_q:
                            op_q.pop(0)()
                            done_op += 1
        if pending is not None:
            emit_O(pending)
        # leftover deferred blocks (old stripes, deps long satisfied) fill
        # the PE while the last unit's normalization chain drains
        while op_q:
            op_q.pop(0)(scalar_evac=True)

    nc.compile()
    return nc


def _get(causal: bool):
    if causal not in _compiled:
        _compiled[causal] = _build(causal)
    return _compiled[causal]


def kernel(q, k, v, mask, w_q, b_q, w_k, b_k, w_v, b_v, w_o, b_o):
    from concourse.bass_utils import run_bass_kernel_spmd

    q = np.asarray(q, dtype=np.float32)
    k = np.asarray(k, dtype=np.float32)
    v = np.asarray(v, dtype=np.float32)
    w_q = np.asarray(w_q, dtype=np.float32)
    w_k = np.asarray(w_k, dtype=np.float32)
    w_v = np.asarray(w_v, dtype=np.float32)
    w_o = np.asarray(w_o, dtype=np.float32)
    b_q = np.asarray(b_q, dtype=np.float32)
    b_k = np.asarray(b_k, dtype=np.float32)
    b_v = np.asarray(b_v, dtype=np.float32)
    b_o = np.asarray(b_o, dtype=np.float32)

    m = np.asarray(mask).reshape(T, T)
    idx = np.arange(T)
    if m.all():
        causal = False
    elif (m == (idx[None, :] <= idx[:, None])).all():
        causal = True
    else:
        raise NotImplementedError("only causal (tril) or full masks supported")

    nc = _get(causal)

    tri_np = np.ascontiguousarray(
        np.asarray(idx[:TK, None] <= idx[None, :TK], dtype=np.float16))

    import ml_dtypes
    f8 = ml_dtypes.float8_e4m3fn
    xq_b = [np.ascontiguousarray(q[b].T.astype(f8)) for b in range(B)]
    xk_b = [np.ascontiguousarray(k[b].T.astype(f8)) for b in range(B)]
    xv_b = [np.ascontiguousarray(v[b].T.astype(np.float16)) for b in range(B)]

    gmaps = []
    for g in range(2):
        sl = slice(g * F, (g + 1) * F)
        gmaps.append({
            "wq": np.ascontiguousarray(w_q[sl, :].T.astype(f8)),
            "wk": np.ascontiguousarray(w_k[sl, :].T.astype(f8)),
            "wv": np.ascontiguousarray(w_v[sl, :].T.astype(np.float16)),
            "wo": np.ascontiguousarray(w_o[:, sl].T.astype(np.float16)),
            "bq": np.ascontiguousarray(b_q[sl].reshape(NF, 128).T),
            "bk": np.ascontiguousarray(b_k[sl].reshape(NF, 128).T),
        })

    in_maps = []
    for c in range(NCORES):
        b, g = c // 2, c % 2
        im = {"xq": xq_b[b], "xk": xk_b[b], "xv": xv_b[b], "tri": tri_np}
        im.update(gmaps[g])
        in_maps.append(im)

    res = run_bass_kernel_spmd(nc, in_maps, core_ids=list(range(NCORES)))

    # constant rows folded out of the device computation
    consts = [b_v[g * F:(g + 1) * F] @ w_o[:, g * F:(g + 1) * F].T
              for g in range(2)]
    add_row = (b_o + consts[0] + consts[1]).astype(np.float32)

    out = np.empty((B, T, D), dtype=np.float32)
    for b in range(B):
        out[b] = (res.results[2 * b]["y"].astype(np.float32)
                  + res.results[2 * b + 1]["y"].astype(np.float32) + add_row)
    return out


# revision 37
# speedup vs baseline: 1.0598x; 1.0587x over previous
"""Multi-head attention (B=4, T=2048, D=1024, H=16 causal) on 8 TRN2 NeuronCores.

Sharding: core c -> (batch b = c // 2, head-group g = c % 2 of 8 heads).
Device computes, per core, in transposed layouts (feature-major):
  qT/kT = (W_{q,k,g} @ X_b^T + b)   -- per 512-col stripe tiles, fp16
  V     = X_b @ W_{v,g}^T           -- (2048, 8, 64) natural layout + 64
                                       ones-cols per head (replicated-Z)
  S^T   = per 128-row k-block vs qT -- both heads of a pair packed as
          concurrent 64x128 row-tiled matmuls (PE rows 0-63 / 64-127)
          into one [128, 2, 512] psum tile (bank A / bank B)
  P~    = exp(S^T/8)  one ACT instr per k-block covering both heads
  [O^T; Z*64] = [V|1x64]^T @ P~     -- ones block gives softmax denom
          replicated on po rows 64..127, so 1/Z comes from a single
          64-lane reciprocal_approx_fast straight off PSUM (no 1-lane
          copies, no GpSimd partition-broadcast)
  O^T  = po[0:64] * rb, then Y_part = O^T-chunks^T @ W_o-slice^T.

Single software pipeline, kt-granular (288us -> target ~250us):
  - score-PSUM tiles are per-k-block and hold BOTH heads, so the two
    64x128 score matmuls are emitted adjacently with identical deps and
    the PE runs them concurrently on disjoint row groups (~2x scores);
    the single-tile release (one exp per k-block) keeps both heads'
    slots freeing together, which is what lets the next block pack too;
  - a ~4us warm-up spinner of tiny matmuls at kernel start trips the
    HAM clock gate to 8/8 before the DMA-bound head delivers real work
    (baseline only warmed at ~33us, paying 2x on all head matmuls);
  - projection bias-evac runs on the DVE (tensor_scalar_add), keeping
    the scalar engine for the ~157us exp stream;
  - output projection for stripes 0..2 is deferred and drained through
    the final stripe as PE filler; the final stripe's own blocks drain
    at the end.
PSUM budget: ss 2x2 (two [128,2,512] slots) + po 2 + pp 2 = 8 banks.
Host: shards/transposes inputs, sums the two per-batch partial Y's,
adds b_o plus the folded V-bias constant row (b_v,g @ W_o,g^T).
"""
import numpy as np
from contextlib import ExitStack

B, T, D = 4, 2048, 1024
H, DK = 16, 64
NCORES = 8
HPC = H // 2            # heads per core
F = HPC * DK            # 512 features per core
SCALE = 1.0 / np.sqrt(DK)
TQ = 512                # q-tile width (free dim)
TK = 128                # k-tile height (partition dim)
NQT = T // TQ           # 4
NKT = T // TK           # 16
ND = D // 128           # 8 contraction chunks for projections
NF = F // 128           # 4 feature chunks per core
PT = 512                # projection t-tile width (== TQ)
NPT = T // PT           # 4

_compiled = {}


def _build(causal: bool):
    import concourse.tile as tile
    from concourse import bacc, mybir

    dt = mybir.dt
    AF = mybir.ActivationFunctionType
    ALU = mybir.AluOpType

    nc = bacc.Bacc("TRN2", target_bir_lowering=False, debug=False,
                   num_devices=NCORES)

    xq = nc.dram_tensor("xq", [D, T], dt.float8e4, kind="ExternalInput")
    xk = nc.dram_tensor("xk", [D, T], dt.float8e4, kind="ExternalInput")
    xv = nc.dram_tensor("xv", [D, T], dt.float16, kind="ExternalInput")
    wq = nc.dram_tensor("wq", [D, F], dt.float8e4, kind="ExternalInput")
    wk = nc.dram_tensor("wk", [D, F], dt.float8e4, kind="ExternalInput")
    wv = nc.dram_tensor("wv", [D, F], dt.float16, kind="ExternalInput")
    wo = nc.dram_tensor("wo", [F, D], dt.float16, kind="ExternalInput")
    bq = nc.dram_tensor("bq", [128, NF], dt.float32, kind="ExternalInput")
    bk = nc.dram_tensor("bk", [128, NF], dt.float32, kind="ExternalInput")
    tri = nc.dram_tensor("tri", [128, 128], dt.float16, kind="ExternalInput")
    y = nc.dram_tensor("y", [T, D], dt.float16, kind="ExternalOutput")

    with tile.TileContext(nc) as tc, ExitStack() as ctx:
        per = ctx.enter_context(tc.tile_pool(name="persist", bufs=1))

        # per-stripe persistent tiles -> fine-grained scheduler deps
        qT = [[per.tile([128, PT], dt.float16, tag=f"qT{f}_{t}",
                        name=f"qT{f}_{t}")
               for t in range(NPT)] for f in range(NF)]
        kT = [[per.tile([128, PT], dt.float16, tag=f"kT{f}_{t}",
                        name=f"kT{f}_{t}")
               for t in range(NPT)] for f in range(NF)]
        # V with 64 leading ones-cols per head: PV output rows 0..63 = Z
        # (replicated), rows 64..127 = O^T; recip needs partition base 0
        vS = [per.tile([128, HPC, 2 * DK], dt.float16, tag=f"v{t}",
                       name=f"v{t}")
              for t in range(NKT)]
        oT = [[per.tile([128, TQ], dt.float16, tag=f"oT{f}_{t}",
                        name=f"oT{f}_{t}")
               for t in range(NQT)] for f in range(NF)]
        bq_sb = per.tile([128, NF], dt.float32, tag="bq")
        bk_sb = per.tile([128, NF], dt.float32, tag="bk")
        tri_sb = per.tile([128, 128], dt.float16, tag="tri")
        warm_sb = per.tile([128, 128], dt.float16, tag="warm")

        # q/k weights in one tile, DMA'd in f-column slices so the first
        # attention unit's chain (wq_f0, xq, wk_f0, xk) is ~2.5MB instead
        # of 4MB; v weights in two halves (512KB per issue)
        wq_sb = per.tile([128, ND, F], dt.float8e4, tag="wq", name="wq")
        wk_sb = per.tile([128, ND, F], dt.float8e4, tag="wk", name="wk")
        wv_sb = per.tile([128, ND, F], dt.float16, tag="wv", name="wv")
        wo_sb = per.tile([128, NF, D], dt.float16, tag="wo")

        wq_re = wq.ap().rearrange("(c p) f -> p c f", p=128)
        wk_re = wk.ap().rearrange("(c p) f -> p c f", p=128)
        wv_re = wv.ap().rearrange("(c p) f -> p c f", p=128)
        xq_re = xq.ap().rearrange("(c p) t -> p c t", p=128)
        xk_re = xk.ap().rearrange("(c p) t -> p c t", p=128)
        xv_re = xv.ap().rearrange("(c p) t -> p c t", p=128)

        px = ctx.enter_context(tc.tile_pool(name="px", bufs=2))
        pps = ctx.enter_context(tc.tile_pool(name="pps", bufs=2, space="PSUM"))
        pa = ctx.enter_context(tc.tile_pool(name="pa", bufs=3))
        pn = ctx.enter_context(tc.tile_pool(name="pn", bufs=2))
        sps = ctx.enter_context(tc.tile_pool(name="sps", bufs=2, space="PSUM"))
        ops = ctx.enter_context(tc.tile_pool(name="ops", bufs=1, space="PSUM"))

        # ---- PE warm-up spinner -----------------------------------------
        # ~4us of continuous N=128 matmuls starting right after the ~7.2us
        # engine-init preamble, so the HAM clock gate reaches 8/8 at
        # ~11.5us (instead of ~33us) while the DMA-bound head streams in;
        # 16 disjoint psum regions so Tile inserts no serializing WAW deps.
        nc.vector.memset(warm_sb[:], 0.0)
        warm_ps = pps.tile([128, PT], dt.float32, tag="pp", name="warm")
        for i in range(44):
            # same col position for all (serial ~107ns cadence, NOT
            # col-group packed) so the spin actually spans ~4.7us
            r = 128 * (i % 4)
            nc.tensor.matmul(warm_ps[0:32, r:r + 128],
                             warm_sb[:, 0:32], warm_sb[:, 0:128],
                             start=True, stop=True)

        # ---- projection sub-tasks ----------------------------------------
        def qk_dma(x_re, t):
            xt = px.tile([128, ND, PT], dt.float8e4, tag="xt", name="xt",
                         bufs=2)
            nc.sync.dma_start(xt[:], x_re[:, :, t * PT:(t + 1) * PT])
            return xt

        def qk_chunk(getx, w_sb, b_sb, dest, t, f):
            # fp8 DoubleRow: 4 K=256 matmuls (2 elem/cell/cycle) replace
            # 8 K=128 fp16 matmuls -- ~2x projection throughput
            ps = pps.tile([128, PT], dt.float32, tag="pp", name="pp")
            for dp in range(ND // 2):
                nc.tensor.matmul(
                    ps[:], w_sb[:, 2 * dp:2 * dp + 2, f * 128:(f + 1) * 128],
                    getx(dp), start=(dp == 0), stop=(dp == ND // 2 - 1),
                    perf_mode=mybir.MatmulPerfMode.DoubleRow)
            # bias-add on DVE keeps the scalar engine free for exp
            nc.vector.tensor_scalar_add(dest[f][t][:], ps[:],
                                        b_sb[:, f:f + 1])

        def v_tile(ts):
            xt = px.tile([128, ND, TK], dt.float16, tag="xtv", name="xtv",
                         bufs=3)
            nc.sync.dma_start(xt[:], xv_re[:, :, ts * TK:(ts + 1) * TK])
            ps = pps.tile([128, F], dt.float32, tag="pp", name="pp")
            for d in range(ND):
                nc.tensor.matmul(ps[:], xt[:, d, :],
                                 wv_sb[:, d, :],
                                 start=(d == 0), stop=(d == ND - 1))
            nc.vector.tensor_copy(
                vS[ts][:, :, DK:2 * DK],
                ps[:].rearrange("p (h e) -> p h e", h=HPC))

        # ---- prologue: minimal stripe-0 work for the first attention unit
        # the head is HBM-transfer-bound: issue strictly in consumption
        # order (q/k halves interleaved so the chains pipeline with the
        # transfers), >=512KB per issue; biases (needed only at evac) and
        # v data come after.  All on the sync queue — DMA issues on the
        # scalar queue would block exp behind them.
        for ts in range(NKT):
            nc.vector.memset(vS[ts][:, :, 0:DK], 1.0)
        xq0 = [px.tile([128, ND // 2, PT], dt.float8e4, tag=f"xq0_{g}",
                       name=f"xq0_{g}") for g in range(2)]
        xk0 = [px.tile([128, ND // 2, PT], dt.float8e4, tag=f"xk0_{g}",
                       name=f"xk0_{g}") for g in range(2)]
        # chain to the first attention unit: wq_f0, xq halves, wk_f0, xk
        # halves (~2.5MB -> first exp at ~16us instead of ~27); then v
        # weights + stripe-0 v slices, then the remaining weight f-slices
        # in consumption order
        nc.sync.dma_start(wq_sb[:, :, 0:128], wq_re[:, :, 0:128])
        nc.sync.dma_start(xq0[0][:], xq_re[:, 0:4, 0:PT])
        nc.sync.dma_start(xq0[1][:], xq_re[:, 4:8, 0:PT])
        nc.sync.dma_start(bq_sb[:], bq.ap())
        nc.sync.dma_start(wk_sb[:, :, 0:128], wk_re[:, :, 0:128])
        nc.sync.dma_start(xk0[0][:], xk_re[:, 0:4, 0:PT])
        nc.sync.dma_start(xk0[1][:], xk_re[:, 4:8, 0:PT])
        nc.sync.dma_start(bk_sb[:], bk.ap())
        if causal:
            nc.sync.dma_start(tri_sb[:], tri.ap())
        # remaining q/k weight f-slices BEFORE the v weights: stripe-0
        # f1..f3 projection chunks are the only PE work available while
        # the v-chain streams, and they keep the clock gate warm
        for f in range(1, NF):
            nc.sync.dma_start(wq_sb[:, :, f * 128:(f + 1) * 128],
                              wq_re[:, :, f * 128:(f + 1) * 128])
            nc.sync.dma_start(wk_sb[:, :, f * 128:(f + 1) * 128],
                              wk_re[:, :, f * 128:(f + 1) * 128])
        for g in range(2):
            nc.sync.dma_start(wv_sb[:, 4 * g:4 * g + 4, :],
                              wv_re[:, 4 * g:4 * g + 4])

        def x0pair(x0):
            return lambda dp: x0[dp // 2][:, 2 * (dp % 2):2 * (dp % 2) + 2, :]

        qk_chunk(x0pair(xq0), wq_sb, bq_sb, qT, 0, 0)
        qk_chunk(x0pair(xk0), wk_sb, bk_sb, kT, 0, 0)

        # remaining stripe-0 q/k chunks + v tiles: drained inside qt0 (v
        # tiles last -- their data arrives latest and PV is lagged past
        # them, so their DMA wait never heads the PE queue)
        fill_own0 = []
        for f in range(1, NF):
            fill_own0.append(lambda f=f: qk_chunk(
                x0pair(xq0), wq_sb, bq_sb, qT, 0, f))
            fill_own0.append(lambda f=f: qk_chunk(
                x0pair(xk0), wk_sb, bk_sb, kT, 0, f))
        for ts in range(4):
            fill_own0.append(lambda ts=ts: v_tile(ts))

        # fill tasks for stripe t: fq (q proj, needed when attention(t)
        # starts) and fkv (k/v proj, needed only by attention(t)'s
        # diagonal blocks, which run last)
        def make_fq(t):
            tasks = []
            state = {}
            def qd():
                state['qxt'] = qk_dma(xq_re, t)
            tasks.append(qd)
            for f in range(NF):
                tasks.append(lambda f=f: qk_chunk(
                    lambda dp: state['qxt'][:, 2 * dp:2 * dp + 2, :],
                    wq_sb, bq_sb, qT, t, f))
            return tasks

        def make_fkv(t):
            tasks = []
            state = {}
            def kd():
                state['kxt'] = qk_dma(xk_re, t)
            tasks.append(kd)
            for f in range(NF):
                tasks.append(lambda f=f: qk_chunk(
                    lambda dp: state['kxt'][:, 2 * dp:2 * dp + 2, :],
                    wk_sb, bk_sb, kT, t, f))
                tasks.append(lambda ts=4 * t + f: v_tile(ts))
            return tasks

        # ---- attention: kt-granular units, heads packed -------------------
        def emit_S_exp(qt, c, kt, oo, w):
            # both heads' 64x128 score matmuls into one [128, 2, TQ] psum
            # tile (h0 -> bank A cols, h1 -> bank B cols): identical deps,
            # adjacent issue, disjoint PE row groups -> they pack
            ss = sps.tile([128, 2, TQ], dt.float32, tag="ss", name="ss")
            for par in range(2):
                base = par * DK
                nc.tensor.matmul(
                    ss[:, par, oo:TQ],
                    kT[c][kt // 4][base:base + DK,
                                   (kt % 4) * TK:(kt % 4 + 1) * TK],
                    qT[c][qt][base:base + DK, oo:TQ],
                    start=True, stop=True)
            pt = pa.tile([128, 2, TQ], dt.float16, tag="pt", name="pt",
                         bufs=12)
            nc.scalar.activation(pt[:, :, oo:TQ], ss[:, :, oo:TQ],
                                 AF.Exp, scale=float(SCALE))
            if causal and oo + TK <= TQ and kt >= 4 * qt:
                for par in range(2):
                    nc.vector.tensor_tensor(
                        pt[:, par, oo:oo + TK], pt[:, par, oo:oo + TK],
                        tri_sb[:], op=ALU.mult)
            return pt

        def emit_O(unit):
            qt, c, kt, oo, pt, first, last, po = unit
            for par in range(2):
                h = 2 * c + par
                nc.tensor.matmul(
                    po[par][:, oo:TQ], vS[kt][:, h, :],
                    pt[:, par, oo:TQ],
                    start=first, stop=last)
            if last:
                for par in range(2):
                    base = par * DK
                    # ones block is FIRST in vS, so Z lands on partitions
                    # 0:63 (reciprocal_approx_fast only works at base 0)
                    # and O on 64:127 (cross-base tensor_tensor is fine)
                    rb = pn.tile([DK, TQ], dt.float32, tag=f"rb{par}",
                                 name=f"rb{par}")
                    nc.vector.reciprocal_approx_fast(
                        out=rb[:], in_=po[par][0:DK, :])
                    nc.vector.tensor_tensor(
                        oT[c][qt][base:base + DK, :],
                        po[par][DK:2 * DK, :], rb[:], op=ALU.mult)
                if c == NF - 1:
                    # defer: output projection is the PE filler that keeps
                    # the clock gate warm through the ACT-bound final
                    # stripe; the last stripe's own blocks also go through
                    # the queue so held-back (dependency-free) blocks drain
                    # first and cover the final normalization-chain wait
                    for tsl in range(TQ // 128):
                        op_q.append(
                            lambda qt=qt, tsl=tsl, **kw:
                            emit_op_block(qt, tsl, **kw))

        def emit_op_block(qt, tsl, scalar_evac=False):
            ts = qt * (TQ // 128) + tsl
            yst = pa.tile([128, D], dt.float16, tag="yst", name="yst")
            for mh in range(2):
                ps = pps.tile([128, 512], dt.float32,
                              tag="pp", name=f"yp{mh}")
                for fc in range(NF):
                    nc.tensor.matmul(
                        ps[:],
                        oT[fc][qt][:, tsl * 128:(tsl + 1) * 128],
                        wo_sb[:, fc, mh * 512:(mh + 1) * 512],
                        start=(fc == 0), stop=(fc == NF - 1))
                if scalar_evac and mh == 0:
                    # final-flush blocks: ACT is idle once the exp stream
                    # ends; splitting evac between ScE and DVE removes the
                    # psum-slot stall that was re-throttling the clock gate
                    nc.scalar.copy(yst[:, mh * 512:(mh + 1) * 512], ps[:])
                else:
                    nc.vector.tensor_copy(yst[:, mh * 512:(mh + 1) * 512],
                                          ps[:])
            nc.sync.dma_start(y.ap()[ts * 128:(ts + 1) * 128, :],
                              yst[:])

        from collections import deque
        pend = deque()
        op_q = []
        # fill draining: during attention(qt) run all of stripe qt+1's
        # projection work (every head-chunk c runs its diagonal blocks, so
        # stripe tiles must be complete before attention(qt+1) starts);
        # deferred output-projection blocks drain through the last stripe
        # as PE filler against the clock-gate.
        for qt in range(NQT):
            if qt + 1 < NPT:
                fill_b = make_fq(qt + 1) + make_fkv(qt + 1)
                if qt == 0:
                    # wo (1MB) is only needed by the deferred output
                    # projection in qt3 -- keep it BEHIND stripe-1 data
                    fill_b.append(lambda: nc.sync.dma_start(
                        wo_sb[:],
                        wo.ap().rearrange("(c p) m -> p c m", p=128)))
            else:
                fill_b = []
            fill_a = fill_own0 if qt == 0 else []
            if causal:
                # regular k-blocks first: the diagonal blocks need stripe
                # qt's k/v tiles, which may still be projecting (fill_a)
                kts = [(kt, 0) for kt in range(4 * qt)] + \
                      [(4 * qt + j, j * TK) for j in range(4)]
            else:
                kts = [(kt, 0) for kt in range(NKT)]
            n_kts = len(kts)
            n_units = NF * n_kts
            n_units_a = NF * max(n_kts - 3, 1)
            done_u = 0
            done_a = 0
            done_b = 0
            done_op = 0
            for c in range(NF):
                po = {}
                for par in range(2):
                    po[par] = ops.tile([128, TQ], dt.float32,
                                       tag=f"po{par}", name=f"po{par}")
                for ui, (kt, oo) in enumerate(kts):
                    pt = emit_S_exp(qt, c, kt, oo, TQ - oo)
                    unit = (qt, c, kt, oo, pt, ui == 0, ui == n_kts - 1,
                            po)
                    pend.append(unit)
                    # PV lag: deep in qt0 so the exp stream never stalls
                    # on late-arriving V data; shallow later (ACT slack)
                    lag = 10 if qt == 0 else 3
                    while len(pend) > lag:
                        emit_O(pend.popleft())
                    done_u += 1
                    want_a = min(len(fill_a), -(-done_u * len(fill_a)
                                                // n_units_a))
                    while done_a < want_a:
                        fill_a[done_a]()
                        done_a += 1
                    want_b = -(-done_u * len(fill_b) // n_units)
                    while done_b < want_b:
                        fill_b[done_b]()
                        done_b += 1
                    if qt == NQT - 1:
                        want_op = done_u * 3 * (TQ // 128) // n_units
                        while done_op < want_op and op_q:
                            op_q.pop(0)()
                            done_op += 1
        while pend:
            emit_O(pend.popleft())
        # leftover deferred blocks (old stripes, deps long satisfied) fill
        # the PE while the last unit's normalization chain drains
        while op_q:
            op_q.pop(0)(scalar_evac=True)

    nc.compile()
    return nc


def _get(causal: bool):
    if causal not in _compiled:
        _compiled[causal] = _build(causal)
    return _compiled[causal]


def kernel(q, k, v, mask, w_q, b_q, w_k, b_k, w_v, b_v, w_o, b_o):
    from concourse.bass_utils import run_bass_kernel_spmd

    q = np.asarray(q, dtype=np.float32)
    k = np.asarray(k, dtype=np.float32)
    v = np.asarray(v, dtype=np.float32)
    w_q = np.asarray(w_q, dtype=np.float32)
    w_k = np.asarray(w_k, dtype=np.float32)
    w_v = np.asarray(w_v, dtype=np.float32)
    w_o = np.asarray(w_o, dtype=np.float32)
    b_q = np.asarray(b_q, dtype=np.float32)
    b_k = np.asarray(b_k, dtype=np.float32)
    b_v = np.asarray(b_v, dtype=np.float32)
    b_o = np.asarray(b_o, dtype=np.float32)

    m = np.asarray(mask).reshape(T, T)
    idx = np.arange(T)
    if m.all():
        causal = False
    elif (m == (idx[None, :] <= idx[:, None])).all():
        causal = True
    else:
        raise NotImplementedError("only causal (tril) or full masks supported")

    nc = _get(causal)

    tri_np = np.ascontiguousarray(
        np.asarray(idx[:TK, None] <= idx[None, :TK], dtype=np.float16))

    import ml_dtypes
    f8 = ml_dtypes.float8_e4m3fn
    xq_b = [np.ascontiguousarray(q[b].T.astype(f8)) for b in range(B)]
    xk_b = [np.ascontiguousarray(k[b].T.astype(f8)) for b in range(B)]
    xv_b = [np.ascontiguousarray(v[b].T.astype(np.float16)) for b in range(B)]

    gmaps = []
    for g in range(2):
        sl = slice(g * F, (g + 1) * F)
        gmaps.append({
            "wq": np.ascontiguousarray(w_q[sl, :].T.astype(f8)),
            "wk": np.ascontiguousarray(w_k[sl, :].T.astype(f8)),
            "wv": np.ascontiguousarray(w_v[sl, :].T.astype(np.float16)),
            "wo": np.ascontiguousarray(w_o[:, sl].T.astype(np.float16)),
            "bq": np.ascontiguousarray(b_q[sl].reshape(NF, 128).T),
            "bk": np.ascontiguousarray(b_k[sl].reshape(NF, 128).T),
        })

    in_maps = []
    for c in range(NCORES):
        b, g = c // 2, c % 2
        im = {"xq": xq_b[b], "xk": xk_b[b], "xv": xv_b[b], "tri": tri_np}
        im.update(gmaps[g])
        in_maps.append(im)

    res = run_bass_kernel_spmd(nc, in_maps, core_ids=list(range(NCORES)))

    # constant rows folded out of the device computation
    consts = [b_v[g * F:(g + 1) * F] @ w_o[:, g * F:(g + 1) * F].T
              for g in range(2)]
    add_row = (b_o + consts[0] + consts[1]).astype(np.float32)

    out = np.empty((B, T, D), dtype=np.float32)
    for b in range(B):
        out[b] = (res.results[2 * b]["y"].astype(np.float32)
                  + res.results[2 * b + 1]["y"].astype(np.float32) + add_row)
    return out


# revision 42
# speedup vs baseline: 1.0752x; 1.0145x over previous
"""Multi-head attention (B=4, T=2048, D=1024, H=16 causal) on 8 TRN2 NeuronCores.

Sharding: core c -> (batch b = c // 2, head-group g = c % 2 of 8 heads).
Device computes, per core, in transposed layouts (feature-major):
  qT/kT = (W_{q,k,g} @ X_b^T + b)   -- per 512-col stripe tiles, fp16
  V     = X_b @ W_{v,g}^T           -- (2048, 8, 64) natural layout + 64
                                       ones-cols per head (replicated-Z)
  S^T   = per 128-row k-block vs qT -- both heads of a pair packed as
          concurrent 64x128 row-tiled matmuls (PE rows 0-63 / 64-127)
          into one [128, 2, 512] psum tile (bank A / bank B)
  P~    = exp(S^T/8)  one ACT instr per k-block covering both heads
  [O^T; Z*64] = [V|1x64]^T @ P~     -- ones block gives softmax denom
          replicated on po rows 64..127, so 1/Z comes from a single
          64-lane reciprocal_approx_fast straight off PSUM (no 1-lane
          copies, no GpSimd partition-broadcast)
  O^T  = po[0:64] * rb, then Y_part = O^T-chunks^T @ W_o-slice^T.

Single software pipeline, kt-granular (288us -> target ~250us):
  - score-PSUM tiles are per-k-block and hold BOTH heads, so the two
    64x128 score matmuls are emitted adjacently with identical deps and
    the PE runs them concurrently on disjoint row groups (~2x scores);
    the single-tile release (one exp per k-block) keeps both heads'
    slots freeing together, which is what lets the next block pack too;
  - a ~4us warm-up spinner of tiny matmuls at kernel start trips the
    HAM clock gate to 8/8 before the DMA-bound head delivers real work
    (baseline only warmed at ~33us, paying 2x on all head matmuls);
  - projection bias-evac runs on the DVE (tensor_scalar_add), keeping
    the scalar engine for the ~157us exp stream;
  - output projection for stripes 0..2 is deferred and drained through
    the final stripe as PE filler; the final stripe's own blocks drain
    at the end.
PSUM budget: ss 2x2 (two [128,2,512] slots) + po 2 + pp 2 = 8 banks.
Host: shards/transposes inputs, sums the two per-batch partial Y's,
adds b_o plus the folded V-bias constant row (b_v,g @ W_o,g^T).
"""
import numpy as np
from contextlib import ExitStack

B, T, D = 4, 2048, 1024
H, DK = 16, 64
NCORES = 8
HPC = H // 2            # heads per core
F = HPC * DK            # 512 features per core
SCALE = 1.0 / np.sqrt(DK)
TQ = 512                # q-tile width (free dim)
TK = 128                # k-tile height (partition dim)
NQT = T // TQ           # 4
NKT = T // TK           # 16
ND = D // 128           # 8 contraction chunks for projections
NF = F // 128           # 4 feature chunks per core
PT = 512                # projection t-tile width (== TQ)
NPT = T // PT           # 4

_compiled = {}


def _build(causal: bool):
    import concourse.tile as tile
    from concourse import bacc, mybir

    dt = mybir.dt
    AF = mybir.ActivationFunctionType
    ALU = mybir.AluOpType

    nc = bacc.Bacc("TRN2", target_bir_lowering=False, debug=False,
                   num_devices=NCORES)

    xq = nc.dram_tensor("xq", [D, T], dt.float8e4, kind="ExternalInput")
    xk = nc.dram_tensor("xk", [D, T], dt.float8e4, kind="ExternalInput")
    xv = nc.dram_tensor("xv", [D, T], dt.float16, kind="ExternalInput")
    wq = nc.dram_tensor("wq", [D, F], dt.float8e4, kind="ExternalInput")
    wk = nc.dram_tensor("wk", [D, F], dt.float8e4, kind="ExternalInput")
    wv = nc.dram_tensor("wv", [D, F], dt.float16, kind="ExternalInput")
    wo = nc.dram_tensor("wo", [F, D], dt.float16, kind="ExternalInput")
    bq = nc.dram_tensor("bq", [128, NF], dt.float32, kind="ExternalInput")
    bk = nc.dram_tensor("bk", [128, NF], dt.float32, kind="ExternalInput")
    tri = nc.dram_tensor("tri", [128, 128], dt.float16, kind="ExternalInput")
    y = nc.dram_tensor("y", [T, D], dt.float16, kind="ExternalOutput")

    with tile.TileContext(nc) as tc, ExitStack() as ctx:
        per = ctx.enter_context(tc.tile_pool(name="persist", bufs=1))

        # per-stripe persistent tiles -> fine-grained scheduler deps
        qT = [[per.tile([128, PT], dt.float16, tag=f"qT{f}_{t}",
                        name=f"qT{f}_{t}")
               for t in range(NPT)] for f in range(NF)]
        kT = [[per.tile([128, PT], dt.float16, tag=f"kT{f}_{t}",
                        name=f"kT{f}_{t}")
               for t in range(NPT)] for f in range(NF)]
        # V with 64 leading ones-cols per head: PV output rows 0..63 = Z
        # (replicated), rows 64..127 = O^T; recip needs partition base 0
        vS = [per.tile([128, HPC, 2 * DK], dt.float16, tag=f"v{t}",
                       name=f"v{t}")
              for t in range(NKT)]
        oT = [[per.tile([128, TQ], dt.float16, tag=f"oT{f}_{t}",
                        name=f"oT{f}_{t}")
               for t in range(NQT)] for f in range(NF)]
        bq_sb = per.tile([128, NF], dt.float32, tag="bq")
        bk_sb = per.tile([128, NF], dt.float32, tag="bk")
        tri_sb = per.tile([128, 128], dt.float16, tag="tri")
        warm_sb = per.tile([128, 128], dt.float16, tag="warm")

        # q/k weights in one tile, DMA'd in f-column slices so the first
        # attention unit's chain (wq_f0, xq, wk_f0, xk) is ~2.5MB instead
        # of 4MB; v weights in two halves (512KB per issue)
        wq_sb = per.tile([128, ND, F], dt.float8e4, tag="wq", name="wq")
        wk_sb = per.tile([128, ND, F], dt.float8e4, tag="wk", name="wk")
        wv_sb = per.tile([128, ND, F], dt.float16, tag="wv", name="wv")
        wo_sb = per.tile([128, NF, D], dt.float16, tag="wo")

        wq_re = wq.ap().rearrange("(c p) f -> p c f", p=128)
        wk_re = wk.ap().rearrange("(c p) f -> p c f", p=128)
        wv_re = wv.ap().rearrange("(c p) f -> p c f", p=128)
        xq_re = xq.ap().rearrange("(c p) t -> p c t", p=128)
        xk_re = xk.ap().rearrange("(c p) t -> p c t", p=128)
        xv_re = xv.ap().rearrange("(c p) t -> p c t", p=128)

        px = ctx.enter_context(tc.tile_pool(name="px", bufs=2))
        pps = ctx.enter_context(tc.tile_pool(name="pps", bufs=2, space="PSUM"))
        pa = ctx.enter_context(tc.tile_pool(name="pa", bufs=3))
        pn = ctx.enter_context(tc.tile_pool(name="pn", bufs=2))
        sps = ctx.enter_context(tc.tile_pool(name="sps", bufs=2, space="PSUM"))
        ops = ctx.enter_context(tc.tile_pool(name="ops", bufs=1, space="PSUM"))

        # ---- PE warm-up spinner -----------------------------------------
        # ~4us of continuous N=128 matmuls starting right after the ~7.2us
        # engine-init preamble, so the HAM clock gate reaches 8/8 at
        # ~11.5us (instead of ~33us) while the DMA-bound head streams in;
        # 16 disjoint psum regions so Tile inserts no serializing WAW deps.
        nc.vector.memset(warm_sb[:], 0.0)
        warm_ps = pps.tile([128, PT], dt.float32, tag="pp", name="warm")
        for i in range(36):
            # same col position for all (serial ~107ns cadence, NOT
            # col-group packed) so the spin actually spans ~4.7us
            r = 128 * (i % 4)
            nc.tensor.matmul(warm_ps[0:32, r:r + 128],
                             warm_sb[:, 0:32], warm_sb[:, 0:128],
                             start=True, stop=True)

        # ---- projection sub-tasks ----------------------------------------
        def qk_dma(x_re, t):
            xt = px.tile([128, ND, PT], dt.float8e4, tag="xt", name="xt",
                         bufs=2)
            nc.sync.dma_start(xt[:], x_re[:, :, t * PT:(t + 1) * PT])
            return xt

        def qk_chunk(getx, w_sb, b_sb, dest, t, f):
            # fp8 DoubleRow: 4 K=256 matmuls (2 elem/cell/cycle) replace
            # 8 K=128 fp16 matmuls -- ~2x projection throughput
            ps = pps.tile([128, PT], dt.float32, tag="pp", name="pp")
            for dp in range(ND // 2):
                nc.tensor.matmul(
                    ps[:], w_sb[:, 2 * dp:2 * dp + 2, f * 128:(f + 1) * 128],
                    getx(dp), start=(dp == 0), stop=(dp == ND // 2 - 1),
                    perf_mode=mybir.MatmulPerfMode.DoubleRow)
            # bias-add on DVE keeps the scalar engine free for exp
            nc.vector.tensor_scalar_add(dest[f][t][:], ps[:],
                                        b_sb[:, f:f + 1])

        def v_tile(ts):
            xt = px.tile([128, ND, TK], dt.float16, tag="xtv", name="xtv",
                         bufs=3)
            nc.sync.dma_start(xt[:], xv_re[:, :, ts * TK:(ts + 1) * TK])
            ps = pps.tile([128, F], dt.float32, tag="pp", name="pp")
            for d in range(ND):
                nc.tensor.matmul(ps[:], xt[:, d, :],
                                 wv_sb[:, d, :],
                                 start=(d == 0), stop=(d == ND - 1))
            nc.vector.tensor_copy(
                vS[ts][:, :, DK:2 * DK],
                ps[:].rearrange("p (h e) -> p h e", h=HPC))

        # ---- prologue: minimal stripe-0 work for the first attention unit
        # the head is HBM-transfer-bound: issue strictly in consumption
        # order (q/k halves interleaved so the chains pipeline with the
        # transfers), >=512KB per issue; biases (needed only at evac) and
        # v data come after.  All on the sync queue — DMA issues on the
        # scalar queue would block exp behind them.
        for ts in range(NKT):
            nc.vector.memset(vS[ts][:, :, 0:DK], 1.0)
        xq0 = px.tile([128, ND, PT], dt.float8e4, tag="xq0", name="xq0")
        xk0 = px.tile([128, ND, PT], dt.float8e4, tag="xk0", name="xk0")
        # chain to the first attention unit: wq_f0, xq, wk_f0, xk (~1.5MB
        # in fp8 -> first exp at ~13us); then the remaining q/k weight
        # f-slices (stripe-0 f1..f3 chunks keep the PE warm while the
        # v-chain streams), then v weights; batched issues -- the sync
        # queue's ~0.6us per-issue cost gates small fp8 transfers
        nc.sync.dma_start(wq_sb[:, :, 0:128], wq_re[:, :, 0:128])
        nc.sync.dma_start(xq0[:], xq_re[:, :, 0:PT])
        nc.sync.dma_start(bq_sb[:], bq.ap())
        nc.sync.dma_start(wk_sb[:, :, 0:128], wk_re[:, :, 0:128])
        nc.sync.dma_start(xk0[:], xk_re[:, :, 0:PT])
        nc.sync.dma_start(bk_sb[:], bk.ap())
        if causal:
            nc.sync.dma_start(tri_sb[:], tri.ap())
        nc.sync.dma_start(wq_sb[:, :, 128:512], wq_re[:, :, 128:512])
        nc.sync.dma_start(wk_sb[:, :, 128:512], wk_re[:, :, 128:512])
        for g in range(2):
            nc.sync.dma_start(wv_sb[:, 4 * g:4 * g + 4, :],
                              wv_re[:, 4 * g:4 * g + 4])

        def x0pair(x0):
            return lambda dp: x0[:, 2 * dp:2 * dp + 2, :]

        qk_chunk(x0pair(xq0), wq_sb, bq_sb, qT, 0, 0)
        qk_chunk(x0pair(xk0), wk_sb, bk_sb, kT, 0, 0)

        # remaining stripe-0 q/k chunks + v tiles: drained inside qt0 (v
        # tiles last -- their data arrives latest and PV is lagged past
        # them, so their DMA wait never heads the PE queue)
        fill_own0 = []
        for f in range(1, NF):
            fill_own0.append(lambda f=f: qk_chunk(
                x0pair(xq0), wq_sb, bq_sb, qT, 0, f))
            fill_own0.append(lambda f=f: qk_chunk(
                x0pair(xk0), wk_sb, bk_sb, kT, 0, f))
        for ts in range(4):
            fill_own0.append(lambda ts=ts: v_tile(ts))

        # fill tasks for stripe t: fq (q proj, needed when attention(t)
        # starts) and fkv (k/v proj, needed only by attention(t)'s
        # diagonal blocks, which run last)
        def make_fq(t):
            tasks = []
            state = {}
            def qd():
                state['qxt'] = qk_dma(xq_re, t)
            tasks.append(qd)
            for f in range(NF):
                tasks.append(lambda f=f: qk_chunk(
                    lambda dp: state['qxt'][:, 2 * dp:2 * dp + 2, :],
                    wq_sb, bq_sb, qT, t, f))
            return tasks

        def make_fkv(t):
            tasks = []
            state = {}
            def kd():
                state['kxt'] = qk_dma(xk_re, t)
            tasks.append(kd)
            for f in range(NF):
                tasks.append(lambda f=f: qk_chunk(
                    lambda dp: state['kxt'][:, 2 * dp:2 * dp + 2, :],
                    wk_sb, bk_sb, kT, t, f))
                tasks.append(lambda ts=4 * t + f: v_tile(ts))
            return tasks

        # ---- attention: kt-granular units, heads packed -------------------
        def emit_S_exp(qt, c, kt, oo, w):
            # both heads' 64x128 score matmuls into one [128, 2, TQ] psum
            # tile (h0 -> bank A cols, h1 -> bank B cols): identical deps,
            # adjacent issue, disjoint PE row groups -> they pack
            ss = sps.tile([128, 2, TQ], dt.float32, tag="ss", name="ss")
            for par in range(2):
                base = par * DK
                nc.tensor.matmul(
                    ss[:, par, oo:TQ],
                    kT[c][kt // 4][base:base + DK,
                                   (kt % 4) * TK:(kt % 4 + 1) * TK],
                    qT[c][qt][base:base + DK, oo:TQ],
                    start=True, stop=True)
            pt = pa.tile([128, 2, TQ], dt.float16, tag="pt", name="pt",
                         bufs=12)
            nc.scalar.activation(pt[:, :, oo:TQ], ss[:, :, oo:TQ],
                                 AF.Exp, scale=float(SCALE))
            if causal and oo + TK <= TQ and kt >= 4 * qt:
                for par in range(2):
                    nc.vector.tensor_tensor(
                        pt[:, par, oo:oo + TK], pt[:, par, oo:oo + TK],
                        tri_sb[:], op=ALU.mult)
            return pt

        def emit_O(unit):
            qt, c, kt, oo, pt, first, last, po = unit
            for par in range(2):
                h = 2 * c + par
                nc.tensor.matmul(
                    po[par][:, oo:TQ], vS[kt][:, h, :],
                    pt[:, par, oo:TQ],
                    start=first, stop=last)
            if last:
                for par in range(2):
                    base = par * DK
                    # ones block is FIRST in vS, so Z lands on partitions
                    # 0:63 (reciprocal_approx_fast only works at base 0)
                    # and O on 64:127 (cross-base tensor_tensor is fine)
                    rb = pn.tile([DK, TQ], dt.float32, tag=f"rb{par}",
                                 name=f"rb{par}")
                    nc.vector.reciprocal_approx_fast(
                        out=rb[:], in_=po[par][0:DK, :])
                    nc.vector.tensor_tensor(
                        oT[c][qt][base:base + DK, :],
                        po[par][DK:2 * DK, :], rb[:], op=ALU.mult)
                if c == NF - 1:
                    # defer: output projection is the PE filler that keeps
                    # the clock gate warm through the ACT-bound final
                    # stripe; the last stripe's own blocks also go through
                    # the queue so held-back (dependency-free) blocks drain
                    # first and cover the final normalization-chain wait
                    for tsl in range(TQ // 128):
                        op_q.append(
                            lambda qt=qt, tsl=tsl, **kw:
                            emit_op_block(qt, tsl, **kw))

        def emit_op_block(qt, tsl, scalar_evac=False):
            ts = qt * (TQ // 128) + tsl
            yst = pa.tile([128, D], dt.float16, tag="yst", name="yst")
            for mh in range(2):
                ps = pps.tile([128, 512], dt.float32,
                              tag="pp", name=f"yp{mh}")
                for fc in range(NF):
                    nc.tensor.matmul(
                        ps[:],
                        oT[fc][qt][:, tsl * 128:(tsl + 1) * 128],
                        wo_sb[:, fc, mh * 512:(mh + 1) * 512],
                        start=(fc == 0), stop=(fc == NF - 1))
                if scalar_evac and mh == 0:
                    # final-flush blocks: ACT is idle once the exp stream
                    # ends; splitting evac between ScE and DVE removes the
                    # psum-slot stall that was re-throttling the clock gate
                    nc.scalar.copy(yst[:, mh * 512:(mh + 1) * 512], ps[:])
                else:
                    nc.vector.tensor_copy(yst[:, mh * 512:(mh + 1) * 512],
                                          ps[:])
            nc.sync.dma_start(y.ap()[ts * 128:(ts + 1) * 128, :],
                              yst[:])

        from collections import deque
        pend = deque()
        op_q = []
        if not causal:
            # full attention touches every k-block from the first q-stripe
            # on: run ALL projections up front (phase-serial, correct;
            # this path is never hit by the tril-masked workload)
            for task in (fill_own0 + make_fq(1) + make_fkv(1)
                         + make_fq(2) + make_fkv(2)
                         + make_fq(3) + make_fkv(3)):
                task()
            nc.sync.dma_start(wo_sb[:],
                              wo.ap().rearrange("(c p) m -> p c m", p=128))
        # fill draining: during attention(qt) run all of stripe qt+1's
        # projection work (every head-chunk c runs its diagonal blocks, so
        # stripe tiles must be complete before attention(qt+1) starts);
        # deferred output-projection blocks drain through the last stripe
        # as PE filler against the clock-gate.
        for qt in range(NQT):
            if causal and qt + 1 < NPT:
                fill_b = make_fq(qt + 1) + make_fkv(qt + 1)
                if qt == 0:
                    # wo (1MB) is only needed by the deferred output
                    # projection in qt3 -- keep it BEHIND stripe-1 data
                    fill_b.append(lambda: nc.sync.dma_start(
                        wo_sb[:],
                        wo.ap().rearrange("(c p) m -> p c m", p=128)))
            else:
                fill_b = []
            fill_a = fill_own0 if (causal and qt == 0) else []
            if causal:
                # regular k-blocks first: the diagonal blocks need stripe
                # qt's k/v tiles, which may still be projecting (fill_a)
                kts = [(kt, 0) for kt in range(4 * qt)] + \
                      [(4 * qt + j, j * TK) for j in range(4)]
            else:
                kts = [(kt, 0) for kt in range(NKT)]
            n_kts = len(kts)
            n_units = NF * n_kts
            n_units_a = NF * max(n_kts - 3, 1)
            done_u = 0
            done_a = 0
            done_b = 0
            done_op = 0
            for c in range(NF):
                po = {}
                for par in range(2):
                    po[par] = ops.tile([128, TQ], dt.float32,
                                       tag=f"po{par}", name=f"po{par}")
                for ui, (kt, oo) in enumerate(kts):
                    pt = emit_S_exp(qt, c, kt, oo, TQ - oo)
                    unit = (qt, c, kt, oo, pt, ui == 0, ui == n_kts - 1,
                            po)
                    pend.append(unit)
                    # PV lag: deep in qt0 so the exp stream never stalls
                    # on late-arriving V data; shallow later (ACT slack);
                    # minimal for the very last block so the tail chain
                    # isn't serialized after the final exp
                    if qt == 0:
                        lag = 10
                    elif qt == NQT - 1 and c == NF - 1:
                        lag = 1
                    else:
                        lag = 3
                    while len(pend) > lag:
                        emit_O(pend.popleft())
                    done_u += 1
                    want_a = min(len(fill_a), -(-done_u * len(fill_a)
                                                // n_units_a))
                    while done_a < want_a:
                        fill_a[done_a]()
                        done_a += 1
                    want_b = -(-done_u * len(fill_b) // n_units)
                    while done_b < want_b:
                        fill_b[done_b]()
                        done_b += 1
                    if qt == NQT - 1:
                        want_op = done_u * 3 * (TQ // 128) // n_units
                        while done_op < want_op and op_q:
                            op_q.pop(0)()
                            done_op += 1
        while pend:
            emit_O(pend.popleft())
        # leftover deferred blocks (old stripes, deps long satisfied) fill
        # the PE while the last unit's normalization chain drains
        while op_q:
            op_q.pop(0)(scalar_evac=True)

    nc.compile()
    return nc


def _get(causal: bool):
    if causal not in _compiled:
        _compiled[causal] = _build(causal)
    return _compiled[causal]


def kernel(q, k, v, mask, w_q, b_q, w_k, b_k, w_v, b_v, w_o, b_o):
    from concourse.bass_utils import run_bass_kernel_spmd

    q = np.asarray(q, dtype=np.float32)
    k = np.asarray(k, dtype=np.float32)
    v = np.asarray(v, dtype=np.float32)
    w_q = np.asarray(w_q, dtype=np.float32)
    w_k = np.asarray(w_k, dtype=np.float32)
    w_v = np.asarray(w_v, dtype=np.float32)
    w_o = np.asarray(w_o, dtype=np.float32)
    b_q = np.asarray(b_q, dtype=np.float32)
    b_k = np.asarray(b_k, dtype=np.float32)
    b_v = np.asarray(b_v, dtype=np.float32)
    b_o = np.asarray(b_o, dtype=np.float32)

    m = np.asarray(mask).reshape(T, T)
    idx = np.arange(T)
    if m.all():
        causal = False
    elif (m == (idx[None, :] <= idx[:, None])).all():
        causal = True
    else:
        raise NotImplementedError("only causal (tril) or full masks supported")

    nc = _get(causal)

    tri_np = np.ascontiguousarray(
        np.asarray(idx[:TK, None] <= idx[None, :TK], dtype=np.float16))

    import ml_dtypes
    f8 = ml_dtypes.float8_e4m3fn
    xq_b = [np.ascontiguousarray(q[b].T.astype(f8)) for b in range(B)]
    xk_b = [np.ascontiguousarray(k[b].T.astype(f8)) for b in range(B)]
    xv_b = [np.ascontiguousarray(v[b].T.astype(np.float16)) for b in range(B)]

    gmaps = []
    for g in range(2):
        sl = slice(g * F, (g + 1) * F)
        gmaps.append({
            "wq": np.ascontiguousarray(w_q[sl, :].T.astype(f8)),
            "wk": np.ascontiguousarray(w_k[sl, :].T.astype(f8)),
            "wv": np.ascontiguousarray(w_v[sl, :].T.astype(np.float16)),
            "wo": np.ascontiguousarray(w_o[:, sl].T.astype(np.float16)),
            "bq": np.ascontiguousarray(b_q[sl].reshape(NF, 128).T),
            "bk": np.ascontiguousarray(b_k[sl].reshape(NF, 128).T),
        })

    in_maps = []
    for c in range(NCORES):
        b, g = c // 2, c % 2
        im = {"xq": xq_b[b], "xk": xk_b[b], "xv": xv_b[b], "tri": tri_np}
        im.update(gmaps[g])
        in_maps.append(im)

    res = run_bass_kernel_spmd(nc, in_maps, core_ids=list(range(NCORES)))

    # constant rows folded out of the device computation
    consts = [b_v[g * F:(g + 1) * F] @ w_o[:, g * F:(g + 1) * F].T
              for g in range(2)]
    add_row = (b_o + consts[0] + consts[1]).astype(np.float32)

    out = np.empty((B, T, D), dtype=np.float32)
    for b in range(B):
        out[b] = (res.results[2 * b]["y"].astype(np.float32)
                  + res.results[2 * b + 1]["y"].astype(np.float32) + add_row)
    return out
